# revision 3
# baseline (speedup 1.0000x reference)
"""CrossModalAttention Trainium2 kernel.

Sharding: 8 cores = batch(4) x query-half(2). Each core computes 2048 queries
of one batch over all 16 heads; k/v projections are recomputed per query-half
(9% duplicate FLOPs) so there are no collectives and outputs are disjoint.

Per-core pipeline (natural-layout softmax):
  xT,sT via PE transpose -> f32 projections -> qT,kT,v (bf16, scale folded
  into kT) -> per (head, 128-query tile): scores MM -> ACT Exp eviction with
  accum_out=Z -> DVE in-place normalize (bf16) + mean-accumulate (f32) ->
  DMA-xbar transpose of attn -> av MM (bf16) -> out-proj MM (bf16) ->
  residual + LayerNorm -> DMA out.
"""

import numpy as np
from contextlib import ExitStack

import concourse.bass as bass
import concourse.tile as tile
from concourse import bacc, mybir
from concourse.bass_utils import run_bass_kernel_spmd
from concourse.masks import make_identity

F32 = mybir.dt.float32
BF16 = mybir.dt.bfloat16

P = 128
NQL = 2048          # queries per core
NS = 1024           # style tokens (keys)
CD = 1024           # content dim
SD = 768            # style dim
H = 16              # heads
D = 64              # head dim
INNER = H * D       # 1024
SCALE = D ** -0.5   # folded into kT eviction
EPS = 1e-5

NQT = NQL // P      # 16 query tiles
NKB = NS // P       # 8 key blocks
NIB = INNER // P    # 8 inner blocks
NCB = CD // P       # 8 content blocks
NSB = SD // P       # 6 style blocks
QG = 4              # query tiles per group (512 queries)
NQG = NQT // QG     # 4 groups


def _bcast_ap(vec_ap: bass.AP, parts: int = P) -> bass.AP:
    # Replicate a 1-D DRAM vector across partitions via a step-0 partition dim.
    return bass.AP(
        tensor=vec_ap.tensor,
        offset=vec_ap.offset,
        ap=[[0, parts]] + list(vec_ap.ap),
    )


def build_kernel(ctx: ExitStack, tc: tile.TileContext, io: dict):
    nc = tc.nc

    x_d, s_d = io["x"], io["s"]
    wq_d, wk_d, wv_d, wo_d = io["wq"], io["wk"], io["wv"], io["wo"]
    bo_d, gamma_d, beta_d = io["bo"], io["gamma"], io["beta"]
    out_d, amean_d = io["out"], io["amean"]

    const = ctx.enter_context(tc.tile_pool(name="const", bufs=1))
    ident = const.tile([P, P], F32)
    make_identity(nc, ident)

    bo128 = const.tile([P, CD], F32)
    gamma128 = const.tile([P, CD], F32)
    beta128 = const.tile([P, CD], F32)
    nc.gpsimd.dma_start(out=bo128, in_=_bcast_ap(bo_d))
    nc.gpsimd.dma_start(out=gamma128, in_=_bcast_ap(gamma_d))
    nc.gpsimd.dma_start(out=beta128, in_=_bcast_ap(beta_d))

    # Persistent attention-phase operands (bf16).
    qTb = const.tile([P, NIB, NQL], BF16)   # [i%128, ib, q]
    kTb = const.tile([P, NIB, NS], BF16)    # [i%128, ib, key]  (pre-scaled)
    vb = const.tile([P, NKB, INNER], BF16)  # [key%128, kb, i]
    wob = const.tile([P, NIB, CD], BF16)    # [i%128, ib, c]

    wq_r = wq_d.rearrange("(cb p) i -> p cb i", p=P)
    wk_r = wk_d.rearrange("(sb p) i -> p sb i", p=P)
    wv_r = wv_d.rearrange("(sb p) i -> p sb i", p=P)

    # ---- Phase A: xT (f32) via PE transpose ----
    xT, free_xT = tc.tile([P, NCB, NQL], F32, name="xT")
    with tc.tile_pool(name="ph_a", bufs=3) as pa, \
         tc.tile_pool(name="ps_a", bufs=2, space="PSUM") as psa:
        for qt in range(NQT):
            xt_in = pa.tile([P, CD], F32, name="xt_in")
            nc.sync.dma_start(out=xt_in, in_=x_d[qt * P:(qt + 1) * P, :])
            for base in (0, 4):
                pt = psa.tile([P, 4 * P], F32, name="pt")
                for j in range(4):
                    nc.tensor.transpose(
                        pt[:, j * P:(j + 1) * P],
                        xt_in[:, (base + j) * P:(base + j + 1) * P],
                        ident,
                    )
                nc.scalar.copy(
                    xT[:, base:base + 4, qt * P:(qt + 1) * P], pt
                )

    # ---- Phase B: qT = (x @ Wq).T -> bf16 ----
    with tc.tile_pool(name="ph_b", bufs=2) as pb, \
         tc.tile_pool(name="ps_b", bufs=4, space="PSUM") as psb:
        for ib in range(NIB):
            wq_sb = pb.tile([P, NCB, P], F32, name="wq_sb")
            nc.sync.dma_start(out=wq_sb, in_=wq_r[:, :, ib * P:(ib + 1) * P])
            for qc in range(NQL // 512):
                pq = psb.tile([P, 512], F32, name="pq")
                for cb in range(NCB):
                    nc.tensor.matmul(
                        pq,
                        wq_sb[:, cb, :],
                        xT[:, cb, qc * 512:(qc + 1) * 512],
                        start=(cb == 0),
                        stop=(cb == NCB - 1),
                    )
                nc.scalar.copy(qTb[:, ib, qc * 512:(qc + 1) * 512], pq)
    free_xT()

    # ---- Phase C: sT (f32) ----
    sT, free_sT = tc.tile([P, NSB, NS], F32, name="sT")
    with tc.tile_pool(name="ph_c", bufs=3) as pc, \
         tc.tile_pool(name="ps_c", bufs=2, space="PSUM") as psc:
        for kb in range(NKB):
            st_in = pc.tile([P, SD], F32, name="st_in")
            nc.sync.dma_start(out=st_in, in_=s_d[kb * P:(kb + 1) * P, :])
            for base, cnt in ((0, 4), (4, 2)):
                pt = psc.tile([P, 4 * P], F32, name="pt")
                for j in range(cnt):
                    nc.tensor.transpose(
                        pt[:, j * P:(j + 1) * P],
                        st_in[:, (base + j) * P:(base + j + 1) * P],
                        ident,
                    )
                nc.scalar.copy(
                    sT[:, base:base + cnt, kb * P:(kb + 1) * P],
                    pt[:, :cnt * P],
                )

    # ---- Phase D: kT (scaled) and v -> bf16 ----
    with tc.tile_pool(name="ph_d", bufs=2) as pd, \
         tc.tile_pool(name="ps_d", bufs=4, space="PSUM") as psd:
        for ib in range(NIB):
            wk_sb = pd.tile([P, NSB, P], F32, name="wk_sb")
            nc.sync.dma_start(out=wk_sb, in_=wk_r[:, :, ib * P:(ib + 1) * P])
            for nck in range(NS // 512):
                pk = psd.tile([P, 512], F32, name="pk")
                for sb in range(NSB):
                    nc.tensor.matmul(
                        pk,
                        wk_sb[:, sb, :],
                        sT[:, sb, nck * 512:(nck + 1) * 512],
                        start=(sb == 0),
                        stop=(sb == NSB - 1),
                    )
                nc.scalar.mul(kTb[:, ib, nck * 512:(nck + 1) * 512], pk, SCALE)
        for ic in range(INNER // 512):
            wv_sb = pd.tile([P, NSB, 512], F32, name="wv_sb")
            nc.sync.dma_start(out=wv_sb, in_=wv_r[:, :, ic * 512:(ic + 1) * 512])
            for kb in range(NKB):
                pv = psd.tile([P, 512], F32, name="pv")
                for sb in range(NSB):
                    nc.tensor.matmul(
                        pv,
                        sT[:, sb, kb * P:(kb + 1) * P],
                        wv_sb[:, sb, :],
                        start=(sb == 0),
                        stop=(sb == NSB - 1),
                    )
                nc.scalar.copy(vb[:, kb, ic * 512:(ic + 1) * 512], pv)
    free_sT()

    # ---- Phase E: Wo -> bf16 ----
    with tc.tile_pool(name="ph_e", bufs=2) as pe:
        for ib in range(NIB):
            wo_in = pe.tile([P, CD], F32, name="wo_in")
            nc.sync.dma_start(out=wo_in, in_=wo_d[ib * P:(ib + 1) * P, :])
            nc.vector.tensor_copy(wob[:, ib, :], wo_in)

    # ---- Attention + output ----
    att = ctx.enter_context(tc.tile_pool(name="att", bufs=3))
    zp_pool = ctx.enter_context(tc.tile_pool(name="zp", bufs=4))
    mean_pool = ctx.enter_context(tc.tile_pool(name="meanp", bufs=1))
    big = ctx.enter_context(tc.tile_pool(name="big", bufs=2))
    outp = ctx.enter_context(tc.tile_pool(name="outp", bufs=2))
    ps_s = ctx.enter_context(tc.tile_pool(name="ps_s", bufs=2, space="PSUM"))
    ps_av = ctx.enter_context(tc.tile_pool(name="ps_av", bufs=2, space="PSUM"))
    ps_o = ctx.enter_context(tc.tile_pool(name="ps_o", bufs=2, space="PSUM"))

    for qg in range(NQG):
        attnT = big.tile([P, NKB, QG * P], BF16, name="attnT")
        avT = big.tile([P, NIB, QG * P], BF16, name="avT")
        mean_acc = [
            mean_pool.tile([P, NS], F32, name=f"mean_acc{qs}") for qs in range(QG)
        ]
        for h in range(H):
            hp = (h % 2) * D           # partition offset of head h
            hb = h // 2                # inner block of head h
            for qs in range(QG):
                qt = qg * QG + qs
                exp_sb = att.tile([P, NS], BF16, name="exp_sb")
                zp = zp_pool.tile([P, 2], F32, name="zp")
                z = zp_pool.tile([P, 1], F32, name="z")
                rz = zp_pool.tile([P, 1], F32, name="rz")
                for ncs in range(NS // 512):
                    pscore = ps_s.tile([P, 512], F32, name="pscore")
                    nc.tensor.matmul(
                        pscore,
                        qTb[hp:hp + D, hb, qt * P:(qt + 1) * P],
                        kTb[hp:hp + D, hb, ncs * 512:(ncs + 1) * 512],
                        start=True,
                        stop=True,
                    )
                    nc.scalar.activation(
                        exp_sb[:, ncs * 512:(ncs + 1) * 512],
                        pscore,
                        mybir.ActivationFunctionType.Exp,
                        accum_out=zp[:, ncs:ncs + 1],
                    )
                nc.vector.tensor_add(z, zp[:, 0:1], zp[:, 1:2])
                nc.vector.reciprocal(rz, z)
                # normalize in place (bf16), then accumulate into f32 mean
                nc.vector.tensor_scalar_mul(exp_sb, exp_sb, rz)
                if h == 0:
                    nc.vector.tensor_copy(mean_acc[qs], exp_sb)
                else:
                    nc.vector.tensor_add(mean_acc[qs], mean_acc[qs], exp_sb)
                nc.sync.dma_start_transpose(
                    attnT[:, :, qs * P:(qs + 1) * P], exp_sb
                )
            pav = ps_av.tile([D, QG * P], F32, name="pav")
            for kb in range(NKB):
                nc.tensor.matmul(
                    pav,
                    vb[:, kb, h * D:(h + 1) * D],
                    attnT[:, kb, :],
                    start=(kb == 0),
                    stop=(kb == NKB - 1),
                )
            nc.scalar.copy(avT[hp:hp + D, hb, :], pav)

        # out-proj + residual + LayerNorm per query tile
        for qs in range(QG):
            qt = qg * QG + qs
            po = [ps_o.tile([P, 512], F32, name=f"po{cc}") for cc in range(2)]
            for cc in range(2):
                for ib in range(NIB):
                    nc.tensor.matmul(
                        po[cc],
                        avT[:, ib, qs * P:(qs + 1) * P],
                        wob[:, ib, cc * 512:(cc + 1) * 512],
                        start=(ib == 0),
                        stop=(ib == NIB - 1),
                    )
            x_in = outp.tile([P, CD], F32, name="x_in")
            nc.sync.dma_start(out=x_in, in_=x_d[qt * P:(qt + 1) * P, :])
            t = outp.tile([P, CD], F32, name="t")
            for cc in range(2):
                nc.vector.tensor_add(
                    t[:, cc * 512:(cc + 1) * 512],
                    po[cc],
                    x_in[:, cc * 512:(cc + 1) * 512],
                )
            nc.vector.tensor_add(t, t, bo128)
            stats = zp_pool.tile([P, 2, 6], F32, name="stats")
            mv = zp_pool.tile([P, 2], F32, name="mv")
            nc.vector.bn_stats(stats[:, 0, :], t[:, 0:512])
            nc.vector.bn_stats(stats[:, 1, :], t[:, 512:1024])
            nc.vector.bn_aggr(mv, stats)
            veps = zp_pool.tile([P, 1], F32, name="veps")
            rvar = zp_pool.tile([P, 1], F32, name="rvar")
            rstd = zp_pool.tile([P, 1], F32, name="rstd")
            nc.vector.tensor_scalar_add(veps, mv[:, 1:2], EPS)
            nc.vector.reciprocal(rvar, veps)
            nc.scalar.activation(rstd, rvar, mybir.ActivationFunctionType.Sqrt)
            nc.vector.tensor_scalar(
                out=t,
                in0=t,
                scalar1=mv[:, 0:1],
                scalar2=rstd,
                op0=mybir.AluOpType.subtract,
                op1=mybir.AluOpType.mult,
            )
            o_sb = outp.tile([P, CD], F32, name="o_sb")
            nc.vector.tensor_mul(o_sb, t, gamma128)
            nc.vector.tensor_add(o_sb, o_sb, beta128)
            nc.sync.dma_start(out=out_d[qt * P:(qt + 1) * P, :], in_=o_sb)

            m_sb = outp.tile([P, NS], F32, name="m_sb")
            nc.scalar.mul(m_sb, mean_acc[qs], 1.0 / H)
            nc.sync.dma_start(out=amean_d[qt * P:(qt + 1) * P, :], in_=m_sb)


_CACHED = None


def _build():
    global _CACHED
    if _CACHED is not None:
        return _CACHED
    nc = bacc.Bacc("TRN2", target_bir_lowering=False, debug=False, num_devices=8)

    def dram(name, shape, kind):
        return nc.dram_tensor(name, shape, F32, kind=kind).ap()

    io = {
        "x": dram("x", [NQL, CD], "ExternalInput"),
        "s": dram("s", [NS, SD], "ExternalInput"),
        "wq": dram("wq", [CD, INNER], "ExternalInput"),
        "wk": dram("wk", [SD, INNER], "ExternalInput"),
        "wv": dram("wv", [SD, INNER], "ExternalInput"),
        "wo": dram("wo", [INNER, CD], "ExternalInput"),
        "bo": dram("bo", [CD], "ExternalInput"),
        "gamma": dram("gamma", [CD], "ExternalInput"),
        "beta": dram("beta", [CD], "ExternalInput"),
        "out": dram("out", [NQL, CD], "ExternalOutput"),
        "amean": dram("amean", [NQL, NS], "ExternalOutput"),
    }
    with tile.TileContext(nc) as tc:
        with ExitStack() as ctx:
            build_kernel(ctx, tc, io)
    nc.compile()
    _CACHED = nc
    return nc


def kernel(**inputs):
    content = np.ascontiguousarray(inputs["content_features"], dtype=np.float32)
    style = np.ascontiguousarray(inputs["style_features"], dtype=np.float32)
    weights = {
        k: np.ascontiguousarray(inputs[key], dtype=np.float32)
        for k, key in [
            ("wq", "Wq"), ("wk", "Wk"), ("wv", "Wv"), ("wo", "Wo"),
            ("bo", "bo"), ("gamma", "gamma"), ("beta", "beta"),
        ]
    }
    nc = _build()
    in_maps = []
    for core in range(8):
        b, half = core // 2, core % 2
        m = {"x": content[b, half * NQL:(half + 1) * NQL], "s": style[b]}
        m.update(weights)
        in_maps.append(m)
    res = run_bass_kernel_spmd(nc, in_maps, core_ids=list(range(8)))
    out = np.empty((4, 2 * NQL, CD), np.float32)
    amean = np.empty((4, 2 * NQL, NS), np.float32)
    for core in range(8):
        b, half = core // 2, core % 2
        out[b, half * NQL:(half + 1) * NQL] = res.results[core]["out"]
        amean[b, half * NQL:(half + 1) * NQL] = res.results[core]["amean"]
    return out, amean
